# revision 2
# baseline (speedup 1.0000x reference)
"""CODI minibatch loss (segment_reduce) on 8 NeuronCores — v4.

Math identical to v2/v3 (per-label sums + per-label squared sums via one-hot
stationary matmuls; host does the [10]-sized algebra in float64).

Schedule:
  - columns split A|V8|V16; A squared on ACT, V8 on DVE (fp8 1x), V16 shipped
    fp16 and squared on DVE in 2x mode; everything reduced per-label on PE.
  - DMA: per-btile transfers on the sync HWDGE ring, region order
    [v8, a, z16] (DVE's chain starts first, it is the longest); one-hots via
    gpsimd SWDGE.
  - compute: btile-pair instructions mid-stream (amortize ACT's 352-cycle
    init), singles at both ends for pipeline fill/drain.
  - btile 7: ACT squares only a[0:1536]; DVE picks up a[1536:A] as a final
    small fp8 tensor_mul so both engines finish together.
  - evac: sums banks 0-2 + sq bank 5 on ACT, sq banks 3-4 on DVE (separate
    tiles, no false WAW), three staged output DMAs, the last one small.
"""

import numpy as np

NUM_LABELS = 10
B_FULL = 8192
C, H = 20, 256
CH = C * H  # 5120
N_CORES = 8
B_LOCAL = B_FULL // N_CORES  # 1024
N_BTILES = B_LOCAL // 128  # 8
EPS = 1e-8

A_COLS = 2304
V8_COLS = 1024
V16_COLS = CH - A_COLS - V8_COLS  # 1792
A7_ACT = 2048  # btile 7: ACT's share of the a-region (chunks 0-3); DVE takes chunk 4

# squared-chunk -> PSUM slot map (chunk indices: a 0-4, v8 5-6, v16 7-10).
# Ordered so bank 3 holds the earliest-closing chunks (v16, squared first on
# btile 7) and bank 5 the last-closing ones (v8 + the a-tail).
SQ_SLOT = {7: 12, 8: 13, 9: 14, 10: 15, 0: 16, 1: 17, 2: 18, 3: 19, 5: 20, 6: 21, 4: 22}

REGIONS = (("a", A_COLS), ("v8", V8_COLS), ("v16", V16_COLS))


def _chunks():
    out = []
    slot = 0
    for ri, (_, w_reg) in enumerate(REGIONS):
        off = 0
        while off < w_reg:
            w = min(512, w_reg - off)
            out.append((ri, off, w, slot))
            slot += 1
            off += w
    assert slot <= 12, slot
    return out

CHUNKS = _chunks()  # 11 slots: a 5, v8 2, v16 4
SQ_SLOT0 = 12
N_BANKS = 6

_CACHE = {}
LAST_RESULT = None


def _build_nc():
    import concourse.bacc as bacc
    import concourse.mybir as mybir
    import concourse.tile as tile

    f8 = mybir.dt.float8e3
    f16 = mybir.dt.float16
    f32 = mybir.dt.float32

    nc = bacc.Bacc("TRN2", target_bir_lowering=False)
    z8a_in = nc.dram_tensor("z8a", [B_LOCAL, A_COLS], f8, kind="ExternalInput")
    z8v_in = nc.dram_tensor("z8v", [B_LOCAL, V8_COLS], f8, kind="ExternalInput")
    z16_in = nc.dram_tensor("z16", [B_LOCAL, V16_COLS], f16, kind="ExternalInput")
    oh8_in = nc.dram_tensor("oh8", [128, N_BTILES * NUM_LABELS], f8, kind="ExternalInput")
    oh16_in = nc.dram_tensor("oh16", [128, N_BTILES * NUM_LABELS], f16, kind="ExternalInput")
    ev_out = nc.dram_tensor("ev", [128, N_BANKS * 512], f16, kind="ExternalOutput")

    with tile.TileContext(nc) as tc:
        with (
            tc.tile_pool(name="zp", bufs=1) as zp,
            tc.tile_pool(name="qp", bufs=1) as qp,
            tc.tile_pool(name="small", bufs=1) as small,
            tc.tile_pool(name="ps", bufs=1, space="PSUM") as psp,
        ):
            s8a = zp.tile([128, N_BTILES * A_COLS], f8, tag="s8a")
            s8v = zp.tile([128, N_BTILES * V8_COLS], f8, tag="s8v")
            s16 = zp.tile([128, N_BTILES * V16_COLS], f16, tag="s16")
            q8a = qp.tile([128, N_BTILES * A_COLS], f16, tag="q8a")
            q8v = qp.tile([128, N_BTILES * V8_COLS], f16, tag="q8v")
            q16 = qp.tile([128, N_BTILES * V16_COLS], f16, tag="q16")

            oh8 = small.tile([128, N_BTILES * NUM_LABELS], f8)
            oh16 = small.tile([128, N_BTILES * NUM_LABELS], f16)
            zeros = small.tile([128, 512], f8)
            evs = small.tile([128, 4 * 512], f16, tag="evs")   # banks 0-2 + 5
            evq = small.tile([128, 2 * 512], f16, tag="evq")   # banks 3-4
            psum = psp.tile([128, N_BANKS * 512], f32)

            nc.gpsimd.dma_start(oh8[:], oh8_in[:])
            nc.gpsimd.dma_start(oh16[:], oh16_in[:])
            nc.gpsimd.memset(zeros[:], 0.0)

            for b in range(N_BANKS):
                nc.tensor.matmul(
                    psum[:, b * 512 : (b + 1) * 512],
                    zeros[:, 0:128],
                    zeros[:, 0:512],
                    start=True,
                    stop=False,
                    skip_group_check=True,
                )

            slabs = {"a": (s8a, A_COLS, q8a), "v8": (s8v, V8_COLS, q8v), "v16": (s16, V16_COLS, q16)}

            def strip_mm(lhs_oh, rhs, slot, last):
                j = slot % 4
                bank = slot // 4
                nc.tensor.matmul(
                    psum[32 * j : 32 * j + NUM_LABELS,
                         bank * 512 : bank * 512 + rhs.shape[-1]],
                    lhs_oh,
                    rhs,
                    start=False,
                    stop=last,
                    skip_group_check=True,
                    tile_position=(0, 32 * j),
                )

            # z DMAs on the sync HWDGE ring: single btiles for the first two
            # waves (fast pipeline fill), pairs after (desc-gen amortization)
            DMA_WAVES = [(0, 1), (1, 1), (2, 2), (4, 2), (6, 1), (7, 1)]
            for b0, nb in DMA_WAVES:
                r0, r1 = b0 * 128, (b0 + nb) * 128
                nc.sync.dma_start(s8v[:, b0 * V8_COLS : (b0 + nb) * V8_COLS], z8v_in[r0:r1, :])
                nc.sync.dma_start(s8a[:, b0 * A_COLS : (b0 + nb) * A_COLS], z8a_in[r0:r1, :])
                nc.sync.dma_start(s16[:, b0 * V16_COLS : (b0 + nb) * V16_COLS], z16_in[r0:r1, :])

            def sums_mms(b, last):
                oh_b8 = oh8[:, b * NUM_LABELS : (b + 1) * NUM_LABELS]
                oh_b16 = oh16[:, b * NUM_LABELS : (b + 1) * NUM_LABELS]
                for ri, off, w, slot in CHUNKS:
                    name, w_reg = REGIONS[ri]
                    slab, _, _ = slabs[name]
                    c0 = b * w_reg + off
                    strip_mm(oh_b16 if name == "v16" else oh_b8,
                             slab[:, c0 : c0 + w], slot, last)

            def sq_mms(b, last, sel=None):
                oh_b16 = oh16[:, b * NUM_LABELS : (b + 1) * NUM_LABELS]
                for ri, off, w, slot in CHUNKS:
                    if sel is not None and slot not in sel:
                        continue
                    name, w_reg = REGIONS[ri]
                    _, _, qslab = slabs[name]
                    c0 = b * w_reg + off
                    strip_mm(oh_b16, qslab[:, c0 : c0 + w], SQ_SLOT[slot], last)

            # compute waves: singles at the ends, pairs in the middle
            COMPUTE_WAVES = [(0, 1), (1, 1), (2, 2), (4, 2), (6, 1), (7, 1)]
            for b0, nb in COMPUTE_WAVES:
                last_wave = b0 + nb == N_BTILES
                v_sl = slice(b0 * V8_COLS, (b0 + nb) * V8_COLS)
                g_sl = slice(b0 * V16_COLS, (b0 + nb) * V16_COLS)
                if not last_wave:
                    a_sl = slice(b0 * A_COLS, (b0 + nb) * A_COLS)
                    nc.vector.tensor_mul(q8v[:, v_sl], s8v[:, v_sl], s8v[:, v_sl])
                    nc.scalar.activation(
                        q8a[:, a_sl], s8a[:, a_sl], mybir.ActivationFunctionType.Square
                    )
                    nc.vector.tensor_mul(q16[:, g_sl], s16[:, g_sl], s16[:, g_sl])
                    for b in range(b0, b0 + nb):
                        sums_mms(b, False)
                        sq_mms(b, False)
                    continue

                # ----- final btile -----
                b = N_BTILES - 1
                a0 = b * A_COLS
                # sums close first so banks 0-2 evac + ship under the sq work
                sums_mms(b, True)
                # DVE: v16 first (its data lands before the a-region tail),
                # then the small a-tail chunk, then v8.  ACT: a[0:2048].
                nc.vector.tensor_mul(q16[:, g_sl], s16[:, g_sl], s16[:, g_sl])
                nc.scalar.activation(
                    q8a[:, a0 : a0 + A7_ACT],
                    s8a[:, a0 : a0 + A7_ACT],
                    mybir.ActivationFunctionType.Square,
                )
                sq_mms(b, True, sel=(7, 8, 9, 10))  # bank 3 (v16)
                nc.vector.tensor_mul(
                    q8a[:, a0 + A7_ACT : a0 + A_COLS],
                    s8a[:, a0 + A7_ACT : a0 + A_COLS],
                    s8a[:, a0 + A7_ACT : a0 + A_COLS],
                )
                nc.vector.tensor_mul(q8v[:, v_sl], s8v[:, v_sl], s8v[:, v_sl])
                # sums banks evacuate + ship while the sq work drains
                nc.scalar.activation(
                    evs[:, 0:1536], psum[:, 0:1536], mybir.ActivationFunctionType.Copy
                )
                nc.sync.dma_start(ev_out[:, 0:1536], evs[:, 0:1536])
                sq_mms(b, True, sel=(0, 1, 2, 3))  # bank 4 (ACT's a-chunks)
                nc.vector.tensor_copy(evq[:], psum[:, 1536:2560])  # banks 3-4
                nc.sync.dma_start(ev_out[:, 1536:2560], evq[:])
                sq_mms(b, True, sel=(5, 6, 4))  # bank 5 (v8 + a-tail)
                nc.scalar.activation(
                    evs[:, 1536:2048], psum[:, 2560:3072],
                    mybir.ActivationFunctionType.Copy,
                )
                nc.sync.dma_start(ev_out[:, 2560:3072], evs[:, 1536:2048])

    nc.compile()
    return nc


def _get_nc():
    if "nc" not in _CACHE:
        _CACHE["nc"] = _build_nc()
    return _CACHE["nc"]


def _ensure_trace_hook():
    import os
    import sys
    import types

    try:
        import antenv.axon_hooks  # noqa: F401

        return
    except ImportError:
        pass
    try:
        import antenv
        import trn_agent_boot.trn_boot as tb

        hook = tb._ntff_profile_via_ctypes("/opt/axon/libaxon_pjrt.so")
        assert hook is not None
        m = types.ModuleType("antenv.axon_hooks")
        m.get_axon_ntff_profile_hook = lambda: hook
        m.set_axon_ntff_profile_hook = lambda h: None
        sys.modules["antenv.axon_hooks"] = m
        antenv.axon_hooks = m
        import concourse.bass_utils as bu

        bu.upload_artifacts = lambda tmpdir: tmpdir
    except Exception:
        os.environ["BASS_NEVER_TRACE"] = "1"


def kernel(z, labels):
    global LAST_RESULT
    import ml_dtypes
    from concourse.bass_utils import run_bass_kernel_spmd

    _ensure_trace_hook()

    z = np.nan_to_num(np.asarray(z, dtype=np.float32)).reshape(B_FULL, CH)
    labels = np.asarray(labels).astype(np.int64)

    z8a = np.ascontiguousarray(z[:, :A_COLS]).astype(ml_dtypes.float8_e3m4)
    z8v = np.ascontiguousarray(z[:, A_COLS : A_COLS + V8_COLS]).astype(
        ml_dtypes.float8_e3m4
    )
    z16 = np.ascontiguousarray(z[:, A_COLS + V8_COLS :]).astype(np.float16)

    onehot = np.zeros((B_FULL, NUM_LABELS), np.float32)
    onehot[np.arange(B_FULL), labels] = 1.0

    in_maps = []
    for c in range(N_CORES):
        sl = slice(c * B_LOCAL, (c + 1) * B_LOCAL)
        oh = (
            onehot[sl]
            .reshape(N_BTILES, 128, NUM_LABELS)
            .transpose(1, 0, 2)
            .reshape(128, N_BTILES * NUM_LABELS)
        )
        in_maps.append(
            {
                "z8a": np.ascontiguousarray(z8a[sl]),
                "z8v": np.ascontiguousarray(z8v[sl]),
                "z16": np.ascontiguousarray(z16[sl]),
                "oh8": np.ascontiguousarray(oh.astype(ml_dtypes.float8_e3m4)),
                "oh16": np.ascontiguousarray(oh.astype(np.float16)),
            }
        )

    nc = _get_nc()
    res = run_bass_kernel_spmd(nc, in_maps, core_ids=list(range(N_CORES)))
    LAST_RESULT = res

    counts = np.bincount(labels, minlength=NUM_LABELS).astype(np.float64)
    sums = np.zeros((NUM_LABELS, CH), np.float64)
    S2 = np.zeros(NUM_LABELS, np.float64)
    reg_off = np.cumsum([0] + [w for _, w in REGIONS])
    for c in range(N_CORES):
        ev = np.asarray(res.results[c]["ev"]).astype(np.float64)  # [128, 3072]
        for ri, off, w, slot in CHUNKS:
            col0 = reg_off[ri] + off
            j, bank = slot % 4, slot // 4
            sums[:, col0 : col0 + w] += ev[
                32 * j : 32 * j + NUM_LABELS, bank * 512 : bank * 512 + w
            ]
            sq_slot = SQ_SLOT[slot]
            jq, bq = sq_slot % 4, sq_slot // 4
            S2 += ev[
                32 * jq : 32 * jq + NUM_LABELS, bq * 512 : bq * 512 + w
            ].sum(axis=1)

    c_safe = np.maximum(counts, 1.0)
    sse = S2 - (sums * sums).sum(axis=1) / c_safe + counts * CH * (EPS * EPS)
    mse = sse / (c_safe * CH)
    loss = np.where(counts > 0, mse, 0.0).sum()
    return np.float32(loss)
